# revision 37
# baseline (speedup 1.0000x reference)
"""AttentionNeuronLayer Trainium2 kernel.

Strategy: shard obs_dim 512 -> 64 rows per core across 8 cores with zero
collectives (each obs row's LSTM recurrence is independent; only the final
out = w @ s contracts over obs rows, summed on host).

The bound is the serial per-step chain; in steady state each step is
~1521ns: sigma 398 -> [512-col wa-tanh chunk 612, exactly covering the
DVE window] -> tanh(c) 238 -> h16/Whh tail 273.  Everything else hides
inside those windows:

 - W_ih @ x for step t+1 prefills a per-gate-block PSUM bank (start=True)
   during step t's chain; Whh(t+1) closes each bank's accumulation group
   (stop=True), so only 4 matmuls sit on the chain before sigma.
 - One Sigmoid covers all 4 gate blocks ([i,f,g,o], g pre-doubled so
   tanh(g) = 2*sig(2g)-1), reading the 4 banks via a strided 3D AP.
 - The sigmoid outputs and cell state share one SBUF tile (CB) so a
   single paged TensorTensor computes t1 = si*sg2 AND cmid = sf*c in one
   DVE instruction; then one STT and one add produce c.
 - wa = tanh(w) runs as one 512-col Act chunk per step, alpha-gated on
   the step's sigmoid so the 4-deep engine wait queue cannot dispatch it
   early; it ends exactly when cadd's sem arrives, so tanh(c) dispatches
   from a busy engine with no idle-wakeup.
 - A DVE filler copy (ready at cadd, ending just past tanh) lets the h16
   multiply dispatch hot; tiny PE pacer matmuls into unused pk-bank
   columns do the same for the next step's Whh.
 - k-path is fully fp16 off h16; the w pair matmul uses the whole kT
   pair [128,128] as one stationary (halving moving columns); the
   every-32-pairs out staging is split in two, deferred into later DVE
   idle windows, and gated (bypass STT reading the filler tile) so it
   cannot preempt the chain's h16 multiply.

Numerics: recurrence fp32; gates' x-path uses fp16 hi/lo K-folding;
attention operands fp16.  Rel err 1.06e-2; cost-model time 400209ns
(baseline 640806ns).
"""

import sys

sys.path.insert(0, "/opt/trn_rl_repo")

import numpy as np

import concourse.bass as bass
import concourse.tile as tile
from concourse import mybir
from concourse.vector_clock import ScopedClock
from concourse.bass_utils import run_bass_kernel_spmd

OBS_DIM = 512
ACT_DIM = 32
HIDDEN_DIM = 1024
MSG_DIM = 128
POS_EM_DIM = 128
T = 256
NCORES = 8
SH = OBS_DIM // NCORES  # 64 obs rows per core
XCH = 8  # xa3 preload chunks

F32 = mybir.dt.float32
F16 = mybir.dt.float16
AF = mybir.ActivationFunctionType
ALU = mybir.AluOpType

# gate blocks laid out [i, f, g, o]; g row pre-scaled by 2 so one Sigmoid
# covers all four gates (tanh(g) = 2*sigmoid(2g) - 1)
_PERM = [0, 1, 2, 3]

TRACE = [False]  # test.py flips this for the profiled run
LAST_RESULTS = [None]


def _patched_drain_and_barrier(self, tick_clock, wait_clock):
    # This walrus build rejects instructions carrying more than one
    # sync-wait command; Tile's tail drain aggregates one wait per live
    # proc.  Re-emit the waits on individual single-wait NOPs instead.
    nc = self.nc
    carrier = nc.sync.nop(nofuse=True)
    wait_clock.add_sem_waits(carrier.ins, ScopedClock({None: tick_clock.global_clock}))
    si = carrier.ins.sync_info
    waits = list(si.on_wait) if si is not None and si.on_wait else []
    if si is not None:
        carrier.ins.sync_info = mybir.SyncInfo(
            on_wait=[], on_update=list(si.on_update or [])
        )
    for w in waits:
        n2 = nc.sync.nop(nofuse=True)
        n2.ins.sync_info = mybir.SyncInfo(on_wait=[w], on_update=[])
    nc.sync.drain()
    nc.all_engine_barrier()
    popped = nc._tile_sem_poison_stack.pop()
    assert popped is self._sem_poison
    nc.clear_and_free_semaphores(list(self.sems.allocated().values()))
    nc.all_engine_barrier()


tile.TileContext._drain_and_barrier = _patched_drain_and_barrier


def _split_multi_waits(module):
    """This walrus build accepts at most one sync-wait command per
    instruction.  Move excess waits onto same-engine NoOps inserted just
    before the instruction — the engine stream is serial, so gating an
    earlier NoOp on the same conditions is equivalent (DMA triggers are
    issued by their engine in program order, so this holds for DMACopy
    too)."""
    import copy as _copy

    counter = [0]
    new_module = _copy.replace(module, functions=[])
    for function in module.functions:
        new_function = _copy.replace(function, blocks=[])
        new_function.set_allocations_from_list(function.allocations)
        for block in function.blocks:
            new_insts = []
            for inst in block.instructions:
                si = inst.sync_info
                waits = list(si.on_wait) if si is not None and si.on_wait else []
                if len(waits) > 1:
                    for w in waits[:-1]:
                        counter[0] += 1
                        nop = mybir.InstNoOp(
                            engine=inst.engine, name=f"I-ws{counter[0]}"
                        )
                        nop.sync_info = mybir.SyncInfo(on_wait=[w], on_update=[])
                        new_insts.append(nop)
                    inst.sync_info = mybir.SyncInfo(
                        on_wait=[waits[-1]], on_update=list(si.on_update or [])
                    )
                new_insts.append(inst)
            new_function.blocks.append(_copy.replace(block, instructions=new_insts))
        new_module.functions.append(new_function)
    return new_module


_NC_CACHE = {}


def _build_nc(split=True):
    if split in _NC_CACHE:
        return _NC_CACHE[split]
    nc = bass.Bass()
    whhT = nc.declare_dram_parameter("whhT", [POS_EM_DIM, 512], F16, isOutput=False)
    wih3T = nc.declare_dram_parameter("wih3T", [102, 512], F16, isOutput=False)
    wkT = nc.declare_dram_parameter("wkT", [POS_EM_DIM, MSG_DIM], F16, isOutput=False)
    bkr = nc.declare_dram_parameter("bkr", [1, MSG_DIM], F16, isOutput=False)
    qT = nc.declare_dram_parameter("qT", [MSG_DIM, HIDDEN_DIM], F16, isOutput=False)
    xa3 = nc.declare_dram_parameter("xa3", [102, T * SH], F16, isOutput=False)
    xTp = nc.declare_dram_parameter("xTp", [2 * SH, T], F16, isOutput=False)
    # transposed out accumulation: 64 steps (32 pairs x 8 hidden blocks x
    # N=2) fill one (128, 512) PSUM bank; host decodes the layout
    outs = nc.declare_dram_parameter("outs", [T // 64, 128, 512], F32, isOutput=True)

    inv_scale = 1.0 / float(np.sqrt(np.float32(MSG_DIM)))
    XC = T * SH // XCH  # cols per xa3 chunk

    with tile.TileContext(nc) as tc:
        with (
            tc.tile_pool(name="const", bufs=1) as const,
            tc.tile_pool(name="state", bufs=1) as state,
            tc.tile_pool(name="hs", bufs=4) as hsp,
            tc.tile_pool(name="work", bufs=3) as work,
            tc.tile_pool(name="aux", bufs=4) as aux,
            tc.tile_pool(name="kp", bufs=3) as kpp,
            tc.tile_pool(name="wap", bufs=3) as wap,
            tc.tile_pool(name="stg", bufs=2) as stg,
            tc.tile_pool(name="pgk", bufs=1, space="PSUM") as pgkp,
            tc.tile_pool(name="pk", bufs=1, space="PSUM") as pkp,
            tc.tile_pool(name="pw", bufs=2, space="PSUM") as pwp,
            tc.tile_pool(name="po", bufs=1, space="PSUM") as pop,
        ):
            whhT_sb = const.tile([POS_EM_DIM, 512], F16)
            wih3T_sb = const.tile([102, 512], F16)
            wkT_sb = const.tile([POS_EM_DIM, MSG_DIM], F16)
            bkr_sb = const.tile([1, MSG_DIM], F16)
            qT_sb = const.tile([MSG_DIM, HIDDEN_DIM], F16)
            xTp_sb = const.tile([2 * SH, T], F16)
            # DMA priority order: step 0 needs wih3T + the first xa3
            # chunk; whhT at step 1; wkT/bkr at the first kT; qT only at
            # the first w matmul; xTp at the first out matmul
            xa3_sb = [
                const.tile([102, XC], F16, name=f"xa3c{ci}") for ci in range(XCH)
            ]
            nc.sync.dma_start(out=wih3T_sb[:], in_=wih3T[:])
            nc.sync.dma_start(out=xa3_sb[0][:], in_=xa3[:, 0:XC])
            nc.sync.dma_start(out=whhT_sb[:], in_=whhT[:])
            nc.sync.dma_start(out=wkT_sb[:], in_=wkT[:])
            nc.sync.dma_start(out=bkr_sb[:], in_=bkr[:])
            nc.sync.dma_start(out=qT_sb[:], in_=qT[:])
            nc.sync.dma_start(out=xTp_sb[:], in_=xTp[:])
            for ci in range(1, XCH):
                nc.sync.dma_start(
                    out=xa3_sb[ci][:], in_=xa3[:, ci * XC : (ci + 1) * XC]
                )

            def xa3_slice(t):
                ci = (t * SH) // XC
                off = t * SH - ci * XC
                return xa3_sb[ci][:, off : off + SH]

            # CB packs the sigmoid outputs (cols 0:256, [i,f,g,o] blocks)
            # and the persistent cell state cT (cols 256:320) into one tile
            # so a single paged TensorTensor can read [sig_i|sig_f] against
            # [sig_g|cT] (page stride 128) and fuse t1 = si*sg2 with
            # cmid = sf*c into one DVE instruction
            CB = state.tile([POS_EM_DIM, 580], F32)
            cT = CB[:, 256:320]
            ones_sb = const.tile([1, 2 * SH], F16)
            nc.vector.memset(CB[:], 0.0)
            nc.vector.memset(ones_sb[:], 1.0)

            poT = pop.tile([128, 512], F32)
            nc.vector.memset(poT[:], 0.0)
            dum_st = const.tile([128, 1], F32)
            nc.vector.memset(dum_st[:], 0.0)
            # dummy activation: pulls the sigmoid/tanh table load (~1.3us)
            # into the DMA window instead of the first real sigma
            warm = state.tile([128, 1], F32)
            nc.vector.memset(warm[:], 0.0)
            nc.scalar.activation(warm[:], warm[:], AF.Sigmoid)

            # one PSUM tile spanning 4 banks: gate blocks i,f,g,o at col
            # offsets 0/512/1024/1536 (own bank each, so W_ih@x for step
            # t+1 prefills with start=True while Whh(t+1) later closes
            # each bank's accumulation group); kT gets a separate bank
            pgk = pgkp.tile([128, 2048], F32)
            pk_t = pkp.tile([128, 512], F32)

            def blk(b):
                return pgk[:, 512 * b : 512 * b + 64]

            sig_in_view = pgk[:].rearrange("p (b x) -> p b x", b=4)[:, :, 0:64]

            h16bufs = {}
            kbufs = {}
            pwbufs = {}
            wabufs = {}
            pending_so = []

            def emit_so_half(gate=None):
                # the every-32-pairs PSUM->SBUF out staging, split in two
                # and deferred into later DVE idle windows; `gate` (the
                # current step's filler tile, riding the unused in1 of a
                # bypass STT) keeps each half from becoming ready before
                # the chain's h16 multiply, which wins the queue-order tie
                pb, so, half = pending_so.pop(0)
                if gate is None:
                    nc.vector.tensor_copy(
                        so[:, 256 * half : 256 * half + 256],
                        poT[:, 256 * half : 256 * half + 256],
                    )
                else:
                    nc.vector.scalar_tensor_tensor(
                        so[:, 256 * half : 256 * half + 256],
                        poT[:, 256 * half : 256 * half + 256],
                        1.0,
                        gate,
                        ALU.mult,
                        ALU.bypass,
                    )
                if half == 1:
                    nc.sync.dma_start(out=outs[pb, :, :], in_=so[:])

            def emit_wx(t, closed=False):
                xs = xa3_slice(t)
                for b in range(4):
                    nc.tensor.matmul(
                        blk(b),
                        wih3T_sb[:, 128 * b : 128 * b + 128],
                        xs,
                        start=True,
                        stop=closed,
                    )

            def emit_whh(t, p, j):
                h_prev = h16bufs[p - 1][:, SH:] if j == 0 else h16bufs[p][:, 0:SH]
                for b in range(4):
                    nc.tensor.matmul(
                        blk(b),
                        whhT_sb[:, 128 * b : 128 * b + 128],
                        h_prev,
                        start=False,
                        stop=True,
                    )

            def emit_kT(p):
                pk = pk_t[:, 0 : 2 * SH]
                nc.tensor.matmul(pk, wkT_sb[:], h16bufs[p][:], start=True, stop=False)
                nc.tensor.matmul(pk, bkr_sb[:], ones_sb[:], start=False, stop=True)

            def emit_kcopy(p):
                # emitted BEFORE the step's chain ops: it becomes ready
                # (pk sem) during the DVE-idle Whh+sigma window and runs
                # there instead of preempting the c-update chain
                kTp = kpp.tile([MSG_DIM, 2 * SH], F16, name=f"kTp{p}", tag="kTp")
                nc.vector.tensor_copy(kTp[:], pk_t[:, 0 : 2 * SH])
                kbufs[p] = kTp

            def emit_w(p):
                # one [128,128] stationary (both steps' k) per 512-col pass:
                # out rows 0-63 = even step of the pair, 64-127 = odd step
                kTp = kbufs.pop(p)
                halves = []
                for h2 in range(2):
                    pw = pwp.tile([128, 512], F32, name=f"pw{p}h{h2}", tag="pw")
                    nc.tensor.matmul(
                        pw[:],
                        kTp[:],
                        qT_sb[:, 512 * h2 : 512 * h2 + 512],
                        start=True,
                        stop=True,
                    )
                    halves.append(pw)
                pwbufs[p] = halves

            def emit_wa_chunk(p, half, gate=None):
                # `gate` (a [128,1] slice of the current step's sigmoid
                # output) rides the unused leaky-relu alpha operand: Tanh
                # ignores its value, but the read dep keeps this chunk
                # from dispatching ahead of the chain ops in the 4-deep
                # engine wait queue.  The 512-col chunk runs back-to-back
                # after sigma and ends just past cadd's sem, so tanh(c)
                # dispatches from the busy engine with no idle-wakeup.
                if half == 0:
                    wabufs[p] = wap.tile(
                        [128, HIDDEN_DIM], F16, name=f"wa{p}", tag="wa"
                    )
                kw = {} if gate is None else {"alpha": gate}
                nc.scalar.activation(
                    wabufs[p][:, 512 * half : 512 * half + 512],
                    pwbufs[p][half][:],
                    AF.Tanh,
                    scale=inv_scale,
                    **kw,
                )

            def emit_out(p):
                # transposed: out rows = hidden sub-dim (M=128), col pair =
                # the two steps; 32 pairs accumulate in one PSUM bank
                wa = wabufs.pop(p)
                base = 16 * (p % 32)
                for i in range(8):
                    nc.tensor.matmul(
                        poT[:, base + 2 * i : base + 2 * i + 2],
                        wa[:, 128 * i : 128 * i + 128],
                        xTp_sb[:, 2 * p : 2 * p + 2],
                        start=True,
                        stop=True,
                    )
                if p % 32 == 31:
                    so = stg.tile([128, 512], F32, tag="so")
                    pending_so.append((p // 32, so, 0))
                    pending_so.append((p // 32, so, 1))

            emit_wx(0, closed=True)
            for t in range(T):
                j = t % 2  # position within the step pair
                p = t // 2
                if j == 0:
                    h16bufs[p] = hsp.tile(
                        [POS_EM_DIM, 2 * SH], F16, name=f"h16p{p}", tag="h16p"
                    )
                # ---- PE: the four Whh chain matmuls, closing each
                # bank's prefilled accumulation group ----
                if t > 0:
                    emit_whh(t, p, j)
                # ---- attention tail: kT -> kcopy -> w for pair p-1;
                # kcopy lands in the DVE-idle Whh+sigma window, the w
                # matmuls run once the copy's sem arrives ----
                if j == 0 and p >= 1:
                    emit_kT(p - 1)
                    emit_kcopy(p - 1)
                    emit_w(p - 1)
                # ---- Act: one sigmoid for all four gates ----
                sig = CB[:, 0:256]
                sig_out = sig.rearrange("p (b x) -> p b x", b=4)
                nc.scalar.activation(sig_out, sig_in_view, AF.Sigmoid)
                # ---- PE: prefill Wx for t+1; the WAR on sigma(t) holds
                # it until the gates are read, then it runs inside this
                # step's chain window ----
                if t + 1 < T:
                    emit_wx(t + 1)
                # ---- DVE: c update  (tanh(g) = 2*sigmoid(2g) - 1) ----
                # one paged mul: page0 t1 = sig_i*sig_g2, page1 cmid = sig_f*c
                t1cm = work.tile([128, 128], F32, tag="t1cm")
                in0 = CB[:, 0:128].rearrange("p (b x) -> p b x", b=2)
                in1 = CB[:, 128:320].rearrange("p (b x) -> p b x", b=3)[:, 0::2, :]
                nc.vector.tensor_mul(
                    t1cm[:].rearrange("p (b x) -> p b x", b=2), in0, in1
                )
                u = work.tile([128, SH], F32, tag="u")
                nc.vector.scalar_tensor_tensor(
                    u[:], t1cm[:, 0:64], 2.0, sig[:, 0:64], ALU.mult, ALU.subtract
                )
                nc.vector.tensor_add(cT, t1cm[:, 64:128], u[:])
                # ---- Act: lagged wa chunk fills the sigma->tanh gap ----
                if j == 0 and p >= 2:
                    emit_wa_chunk(p - 2, 1, gate=sig[:, 0:1])
                elif j == 1 and p >= 1:
                    emit_wa_chunk(p - 1, 0, gate=sig[:, 0:1])
                # filler copy anchored on the chunk's output: its sem
                # arrives ~100 after the chunk ends (= tanh start), so the
                # filler ends just past tanh regardless of DVE-side timing
                # jitter, and h16 dispatches hot on every step
                fil = aux.tile([128, 340], F16, tag="fil")
                if j == 0 and p >= 2:
                    nc.vector.tensor_copy(fil[:], wabufs[p - 2][:, 512:852])
                elif j == 1 and p >= 1:
                    nc.vector.tensor_copy(fil[:], wabufs[p - 1][:, 0:340])
                else:
                    nc.vector.tensor_copy(fil[:], CB[:, 100:440])
                # ---- Act: tanh(c);  DVE: h16 = sig_o * tanh(c) ----
                tct = work.tile([128, SH], F32, tag="tct")
                nc.scalar.activation(tct[:], cT, AF.Tanh)
                nc.vector.tensor_mul(
                    h16bufs[p][:, SH * j : SH * j + SH], sig[:, 192:256], tct[:]
                )
                # last step: pull pair-126's second chunk out of the
                # epilogue into the (now tail-less) post-tanh window
                if t == T - 1:
                    emit_wa_chunk(p - 1, 1, gate=cT[:, 0:1])
                # ---- tails: out staging halves from earlier boundaries
                # drain first (so a fresh boundary's halves pop one step
                # later, when their poT deps have long arrived) ----
                if pending_so:
                    emit_so_half(fil[:, 0:256])
                if j == 0:
                    if p >= 2:
                        emit_out(p - 2)
                    h16bufs.pop(p - 2, None)
                # ---- PE pacer: dummy matmuls (into unused pk-bank cols)
                # keep PE busy until h16's sem has landed, so Whh(t+1)
                # dispatches without the blocked-redispatch penalty.
                # Gates ride the moving operand: after a j=0 step PE is
                # busy long past cadd, so gate on sigma only (CB[:,0:256])
                # and run long; after a j=1 step PE idles early, so gate
                # on cT (arrives while PE still busy) and run short.
                if t + 1 < T:
                    if j == 1:
                        nc.tensor.matmul(
                            pk_t[0:1, 128:296], dum_st[:], CB[:, 152:320],
                            start=True, stop=True,
                        )
                    else:
                        # PE goes idle right after the out matmuls here; a
                        # tiny pacer bridges to h16's sem arrival
                        nc.tensor.matmul(
                            pk_t[0:1, 128:168], dum_st[:], CB[:, 280:320],
                            start=True, stop=True,
                        )
            # ---- epilogue: drain the attention pipeline ----
            while pending_so:
                emit_so_half()
            last = T // 2 - 1  # 127
            emit_kT(last)
            emit_kcopy(last)
            emit_w(last)
            emit_out(last - 1)
            emit_wa_chunk(last, 0)
            emit_wa_chunk(last, 1)
            emit_out(last)
            while pending_so:
                emit_so_half()
    if split:
        nc.m = _split_multi_waits(nc.m)
    _NC_CACHE[split] = nc
    return nc


def kernel(
    obs,
    prev_act,
    in_shift,
    in_scale,
    pos_embedding,
    W_ih,
    b_ih,
    W_hh,
    b_hh,
    Wq,
    bq,
    Wk,
    bk,
):
    obs = np.asarray(obs, np.float32)
    prev_act = np.asarray(prev_act, np.float32)
    in_shift = np.asarray(in_shift, np.float32)
    in_scale = np.asarray(in_scale, np.float32)
    pos_embedding = np.asarray(pos_embedding, np.float32)
    W_ih = np.asarray(W_ih, np.float32)
    b_ih = np.asarray(b_ih, np.float32)
    W_hh = np.asarray(W_hh, np.float32)
    b_hh = np.asarray(b_hh, np.float32)
    Wq = np.asarray(Wq, np.float32)
    bq = np.asarray(bq, np.float32)
    Wk = np.asarray(Wk, np.float32)
    bk = np.asarray(bk, np.float32)

    x = (obs - in_shift) / (in_scale + 1e-8)  # (T, 512)
    q = pos_embedding @ Wq.T + bq  # (1024, 128)
    qT = np.ascontiguousarray(q.T)  # (128, 1024)

    def blocks(mat_rows):  # reorder gate blocks to [i, f, g, o]
        return np.concatenate([mat_rows[128 * p : 128 * p + 128] for p in _PERM], 0)

    W_ih_r = blocks(W_ih)  # (512, 33)
    W_hh_r = blocks(W_hh)  # (512, 128)
    b_r = blocks((b_ih + b_hh)[:, None])[:, 0]  # (512,)
    # g block (rows 256:384 after reorder) doubled: tanh(g) = 2*sig(2g)-1
    gs = np.ones((512, 1), np.float32)
    gs[256:384] = 2.0
    W_ih_r = W_ih_r * gs
    W_hh_r = W_hh_r * gs
    b_r = b_r * gs[:, 0]

    whhT = np.ascontiguousarray(W_hh_r.T)  # (128, 512)
    wih1T = np.concatenate(
        [W_ih_r[:, 0:1].T, np.ascontiguousarray(W_ih_r[:, 1:33].T), b_r[None, :]], 0
    )  # (34, 512)
    # fp16 hi/lo folded into K: [Whi; Whi; Wlo] x [xhi; xlo; xhi] gives
    # Whi*xhi + Whi*xlo + Wlo*xhi (residual ~2^-22) in one K=102 fp16 MM
    whi = wih1T.astype(np.float16)
    wlo = (wih1T - whi.astype(np.float32)).astype(np.float16)
    wih3T = np.concatenate([whi, whi, wlo], 0)  # (102, 512) fp16
    wkT = np.ascontiguousarray(Wk.T)  # (128, 128)
    bkr = bk[None, :]  # (1, 128)

    nc = _build_nc()
    shared = {
        "whhT": whhT.astype(np.float16),
        "wih3T": np.ascontiguousarray(wih3T),
        "wkT": wkT.astype(np.float16),
        "bkr": np.ascontiguousarray(bkr).astype(np.float16),
        "qT": qT.astype(np.float16),
    }
    in_maps = []
    for c in range(NCORES):
        xs = x[:, c * SH : (c + 1) * SH]  # (T, 64)
        xa1 = np.empty((34, T * SH), np.float32)
        xa1[0] = xs.reshape(-1)
        xa1[1:33] = np.repeat(prev_act.T, SH, axis=1).reshape(32, T * SH)
        xa1[33] = 1.0
        xahi = xa1.astype(np.float16)
        xalo = (xa1 - xahi.astype(np.float32)).astype(np.float16)
        xa3 = np.concatenate([xahi, xalo, xahi], 0)  # (102, T*SH) fp16
        # block-diagonal paired s columns: col t has s_t in rows [64j, 64j+64)
        # for j = t%2, zeros elsewhere
        xTp = np.zeros((2 * SH, T), np.float16)
        xTp[0:SH, 0::2] = xs.T[:, 0::2]
        xTp[SH : 2 * SH, 1::2] = xs.T[:, 1::2]
        in_maps.append({**shared, "xa3": xa3, "xTp": xTp})

    res = run_bass_kernel_spmd(nc, in_maps, list(range(NCORES)), trace=TRACE[0])
    LAST_RESULTS[0] = res
    total = np.zeros((T, HIDDEN_DIM), np.float32)
    for c in range(NCORES):
        raw = res.results[c]["outs"]  # (T//64, 128, 512)
        # col = (pair%32)*16 + hidden_block*2 + step_in_pair
        total += np.transpose(
            raw.reshape(T // 64, 128, 32, 8, 2), (0, 2, 4, 3, 1)
        ).reshape(T, HIDDEN_DIM)
    return np.tanh(total).astype(np.float32)


# revision 38
# speedup vs baseline: 1.0052x; 1.0052x over previous
"""AttentionNeuronLayer Trainium2 kernel.

Strategy: shard obs_dim 512 -> 64 rows per core across 8 cores with zero
collectives (each obs row's LSTM recurrence is independent; only the final
out = w @ s contracts over obs rows, summed on host).

The bound is the serial per-step chain; in steady state each step is
~1521ns: sigma 398 -> [512-col wa-tanh chunk 612, exactly covering the
DVE window] -> tanh(c) 238 -> h16/Whh tail 273.  Everything else hides
inside those windows:

 - W_ih @ x for step t+1 prefills a per-gate-block PSUM bank (start=True)
   during step t's chain; Whh(t+1) closes each bank's accumulation group
   (stop=True), so only 4 matmuls sit on the chain before sigma.
 - One Sigmoid covers all 4 gate blocks ([i,f,g,o], g pre-doubled so
   tanh(g) = 2*sig(2g)-1), reading the 4 banks via a strided 3D AP.
 - The sigmoid outputs and cell state share one SBUF tile (CB) so a
   single paged TensorTensor computes t1 = si*sg2 AND cmid = sf*c in one
   DVE instruction; then one STT and one add produce c.
 - wa = tanh(w) runs as one 512-col Act chunk per step, alpha-gated on
   the step's sigmoid so the 4-deep engine wait queue cannot dispatch it
   early; it ends exactly when cadd's sem arrives, so tanh(c) dispatches
   from a busy engine with no idle-wakeup.
 - A DVE filler copy (ready at cadd, ending just past tanh) lets the h16
   multiply dispatch hot; tiny PE pacer matmuls into unused pk-bank
   columns do the same for the next step's Whh.
 - k-path is fully fp16 off h16; the w pair matmul uses the whole kT
   pair [128,128] as one stationary (halving moving columns); the
   every-32-pairs out staging is split in two, deferred into later DVE
   idle windows, and gated (bypass STT reading the filler tile) so it
   cannot preempt the chain's h16 multiply.

Numerics: recurrence fp32; gates' x-path uses fp16 hi/lo K-folding;
attention operands fp16.  Rel err 1.06e-2; cost-model time 400209ns
(baseline 640806ns).
"""

import sys

sys.path.insert(0, "/opt/trn_rl_repo")

import numpy as np

import concourse.bass as bass
import concourse.tile as tile
from concourse import mybir
from concourse.vector_clock import ScopedClock
from concourse.bass_utils import run_bass_kernel_spmd

OBS_DIM = 512
ACT_DIM = 32
HIDDEN_DIM = 1024
MSG_DIM = 128
POS_EM_DIM = 128
T = 256
NCORES = 8
SH = OBS_DIM // NCORES  # 64 obs rows per core
XCH = 8  # xa3 preload chunks

F32 = mybir.dt.float32
F16 = mybir.dt.float16
AF = mybir.ActivationFunctionType
ALU = mybir.AluOpType

# gate blocks laid out [i, f, g, o]; g row pre-scaled by 2 so one Sigmoid
# covers all four gates (tanh(g) = 2*sigmoid(2g) - 1)
_PERM = [0, 1, 2, 3]

TRACE = [False]  # test.py flips this for the profiled run
LAST_RESULTS = [None]


def _patched_drain_and_barrier(self, tick_clock, wait_clock):
    # This walrus build rejects instructions carrying more than one
    # sync-wait command; Tile's tail drain aggregates one wait per live
    # proc.  Re-emit the waits on individual single-wait NOPs instead.
    nc = self.nc
    carrier = nc.sync.nop(nofuse=True)
    wait_clock.add_sem_waits(carrier.ins, ScopedClock({None: tick_clock.global_clock}))
    si = carrier.ins.sync_info
    waits = list(si.on_wait) if si is not None and si.on_wait else []
    if si is not None:
        carrier.ins.sync_info = mybir.SyncInfo(
            on_wait=[], on_update=list(si.on_update or [])
        )
    for w in waits:
        n2 = nc.sync.nop(nofuse=True)
        n2.ins.sync_info = mybir.SyncInfo(on_wait=[w], on_update=[])
    nc.sync.drain()
    nc.all_engine_barrier()
    popped = nc._tile_sem_poison_stack.pop()
    assert popped is self._sem_poison
    nc.clear_and_free_semaphores(list(self.sems.allocated().values()))
    nc.all_engine_barrier()


tile.TileContext._drain_and_barrier = _patched_drain_and_barrier


def _split_multi_waits(module):
    """This walrus build accepts at most one sync-wait command per
    instruction.  Move excess waits onto same-engine NoOps inserted just
    before the instruction — the engine stream is serial, so gating an
    earlier NoOp on the same conditions is equivalent (DMA triggers are
    issued by their engine in program order, so this holds for DMACopy
    too)."""
    import copy as _copy

    counter = [0]
    new_module = _copy.replace(module, functions=[])
    for function in module.functions:
        new_function = _copy.replace(function, blocks=[])
        new_function.set_allocations_from_list(function.allocations)
        for block in function.blocks:
            new_insts = []
            for inst in block.instructions:
                si = inst.sync_info
                waits = list(si.on_wait) if si is not None and si.on_wait else []
                if len(waits) > 1:
                    for w in waits[:-1]:
                        counter[0] += 1
                        nop = mybir.InstNoOp(
                            engine=inst.engine, name=f"I-ws{counter[0]}"
                        )
                        nop.sync_info = mybir.SyncInfo(on_wait=[w], on_update=[])
                        new_insts.append(nop)
                    inst.sync_info = mybir.SyncInfo(
                        on_wait=[waits[-1]], on_update=list(si.on_update or [])
                    )
                new_insts.append(inst)
            new_function.blocks.append(_copy.replace(block, instructions=new_insts))
        new_module.functions.append(new_function)
    return new_module


_NC_CACHE = {}


def _build_nc(split=True):
    if split in _NC_CACHE:
        return _NC_CACHE[split]
    nc = bass.Bass()
    whhT = nc.declare_dram_parameter("whhT", [POS_EM_DIM, 512], F16, isOutput=False)
    wih3T = nc.declare_dram_parameter("wih3T", [102, 512], F16, isOutput=False)
    wkT = nc.declare_dram_parameter("wkT", [POS_EM_DIM, MSG_DIM], F16, isOutput=False)
    bkr = nc.declare_dram_parameter("bkr", [1, MSG_DIM], F16, isOutput=False)
    qT = nc.declare_dram_parameter("qT", [MSG_DIM, HIDDEN_DIM], F16, isOutput=False)
    xa3 = nc.declare_dram_parameter("xa3", [102, T * SH], F16, isOutput=False)
    xTp = nc.declare_dram_parameter("xTp", [2 * SH, T], F16, isOutput=False)
    # transposed out accumulation: 64 steps (32 pairs x 8 hidden blocks x
    # N=2) fill one (128, 512) PSUM bank; host decodes the layout
    outs = nc.declare_dram_parameter("outs", [T // 64, 128, 512], F32, isOutput=True)

    inv_scale = 1.0 / float(np.sqrt(np.float32(MSG_DIM)))
    XC = T * SH // XCH  # cols per xa3 chunk

    with tile.TileContext(nc) as tc:
        with (
            tc.tile_pool(name="const", bufs=1) as const,
            tc.tile_pool(name="state", bufs=1) as state,
            tc.tile_pool(name="hs", bufs=4) as hsp,
            tc.tile_pool(name="work", bufs=3) as work,
            tc.tile_pool(name="aux", bufs=4) as aux,
            tc.tile_pool(name="kp", bufs=3) as kpp,
            tc.tile_pool(name="wap", bufs=3) as wap,
            tc.tile_pool(name="stg", bufs=2) as stg,
            tc.tile_pool(name="pgk", bufs=1, space="PSUM") as pgkp,
            tc.tile_pool(name="pk", bufs=1, space="PSUM") as pkp,
            tc.tile_pool(name="pw", bufs=2, space="PSUM") as pwp,
            tc.tile_pool(name="po", bufs=1, space="PSUM") as pop,
        ):
            whhT_sb = const.tile([POS_EM_DIM, 512], F16)
            wih3T_sb = const.tile([102, 512], F16)
            wkT_sb = const.tile([POS_EM_DIM, MSG_DIM], F16)
            bkr_sb = const.tile([1, MSG_DIM], F16)
            qT_sb = const.tile([MSG_DIM, HIDDEN_DIM], F16)
            xTp_sb = const.tile([2 * SH, T], F16)
            # DMA priority order: step 0 needs wih3T + the first xa3
            # chunk; whhT at step 1; wkT/bkr at the first kT; qT only at
            # the first w matmul; xTp at the first out matmul
            xa3_sb = [
                const.tile([102, XC], F16, name=f"xa3c{ci}") for ci in range(XCH)
            ]
            nc.sync.dma_start(out=wih3T_sb[:], in_=wih3T[:])
            nc.sync.dma_start(out=xa3_sb[0][:], in_=xa3[:, 0:XC])
            nc.sync.dma_start(out=whhT_sb[:], in_=whhT[:])
            nc.sync.dma_start(out=wkT_sb[:], in_=wkT[:])
            nc.sync.dma_start(out=bkr_sb[:], in_=bkr[:])
            nc.sync.dma_start(out=qT_sb[:], in_=qT[:])
            nc.sync.dma_start(out=xTp_sb[:], in_=xTp[:])
            for ci in range(1, XCH):
                nc.sync.dma_start(
                    out=xa3_sb[ci][:], in_=xa3[:, ci * XC : (ci + 1) * XC]
                )

            def xa3_slice(t):
                ci = (t * SH) // XC
                off = t * SH - ci * XC
                return xa3_sb[ci][:, off : off + SH]

            # CB packs the sigmoid outputs (cols 0:256, [i,f,g,o] blocks)
            # and the persistent cell state cT (cols 256:320) into one tile
            # so a single paged TensorTensor can read [sig_i|sig_f] against
            # [sig_g|cT] (page stride 128) and fuse t1 = si*sg2 with
            # cmid = sf*c into one DVE instruction
            CB = state.tile([POS_EM_DIM, 580], F32)
            cT = CB[:, 256:320]
            ones_sb = const.tile([1, 2 * SH], F16)
            nc.vector.memset(CB[:], 0.0)
            nc.vector.memset(ones_sb[:], 1.0)

            poT = pop.tile([128, 512], F32)
            nc.vector.memset(poT[:], 0.0)
            dum_st = const.tile([128, 1], F32)
            nc.vector.memset(dum_st[:], 0.0)
            # dummy activation: pulls the sigmoid/tanh table load (~1.3us)
            # into the DMA window instead of the first real sigma
            warm = state.tile([128, 1], F32)
            nc.vector.memset(warm[:], 0.0)
            nc.scalar.activation(warm[:], warm[:], AF.Sigmoid)

            # one PSUM tile spanning 4 banks: gate blocks i,f,g,o at col
            # offsets 0/512/1024/1536 (own bank each, so W_ih@x for step
            # t+1 prefills with start=True while Whh(t+1) later closes
            # each bank's accumulation group); kT gets a separate bank
            pgk = pgkp.tile([128, 2048], F32)
            pk_t = pkp.tile([128, 512], F32)

            def blk(b):
                return pgk[:, 512 * b : 512 * b + 64]

            sig_in_view = pgk[:].rearrange("p (b x) -> p b x", b=4)[:, :, 0:64]

            h16bufs = {}
            kbufs = {}
            pwbufs = {}
            wabufs = {}
            pending_so = []

            def emit_so_half(gate=None):
                # the every-32-pairs PSUM->SBUF out staging, split in two
                # and deferred into later DVE idle windows; `gate` (the
                # current step's filler tile, riding the unused in1 of a
                # bypass STT) keeps each half from becoming ready before
                # the chain's h16 multiply, which wins the queue-order tie
                pb, so, half = pending_so.pop(0)
                if gate is None:
                    nc.vector.tensor_copy(
                        so[:, 256 * half : 256 * half + 256],
                        poT[:, 256 * half : 256 * half + 256],
                    )
                else:
                    nc.vector.scalar_tensor_tensor(
                        so[:, 256 * half : 256 * half + 256],
                        poT[:, 256 * half : 256 * half + 256],
                        1.0,
                        gate,
                        ALU.mult,
                        ALU.bypass,
                    )
                if half == 1:
                    nc.sync.dma_start(out=outs[pb, :, :], in_=so[:])

            def emit_wx(t, closed=False):
                xs = xa3_slice(t)
                for b in range(4):
                    nc.tensor.matmul(
                        blk(b),
                        wih3T_sb[:, 128 * b : 128 * b + 128],
                        xs,
                        start=True,
                        stop=closed,
                    )

            def emit_whh(t, p, j):
                h_prev = h16bufs[p - 1][:, SH:] if j == 0 else h16bufs[p][:, 0:SH]
                for b in range(4):
                    nc.tensor.matmul(
                        blk(b),
                        whhT_sb[:, 128 * b : 128 * b + 128],
                        h_prev,
                        start=False,
                        stop=True,
                    )

            def emit_kT(p):
                pk = pk_t[:, 0 : 2 * SH]
                nc.tensor.matmul(pk, wkT_sb[:], h16bufs[p][:], start=True, stop=False)
                nc.tensor.matmul(pk, bkr_sb[:], ones_sb[:], start=False, stop=True)

            def emit_kcopy(p):
                # emitted BEFORE the step's chain ops: it becomes ready
                # (pk sem) during the DVE-idle Whh+sigma window and runs
                # there instead of preempting the c-update chain
                kTp = kpp.tile([MSG_DIM, 2 * SH], F16, name=f"kTp{p}", tag="kTp")
                nc.vector.tensor_copy(kTp[:], pk_t[:, 0 : 2 * SH])
                kbufs[p] = kTp

            def emit_w(p):
                # one [128,128] stationary (both steps' k) per 512-col pass:
                # out rows 0-63 = even step of the pair, 64-127 = odd step
                kTp = kbufs.pop(p)
                halves = []
                for h2 in range(2):
                    pw = pwp.tile([128, 512], F32, name=f"pw{p}h{h2}", tag="pw")
                    nc.tensor.matmul(
                        pw[:],
                        kTp[:],
                        qT_sb[:, 512 * h2 : 512 * h2 + 512],
                        start=True,
                        stop=True,
                    )
                    halves.append(pw)
                pwbufs[p] = halves

            def emit_wa_chunk(p, half, gate=None):
                # `gate` (a [128,1] slice of the current step's sigmoid
                # output) rides the unused leaky-relu alpha operand: Tanh
                # ignores its value, but the read dep keeps this chunk
                # from dispatching ahead of the chain ops in the 4-deep
                # engine wait queue.  The 512-col chunk runs back-to-back
                # after sigma and ends just past cadd's sem, so tanh(c)
                # dispatches from the busy engine with no idle-wakeup.
                if half == 0:
                    wabufs[p] = wap.tile(
                        [128, HIDDEN_DIM], F16, name=f"wa{p}", tag="wa"
                    )
                kw = {} if gate is None else {"alpha": gate}
                nc.scalar.activation(
                    wabufs[p][:, 512 * half : 512 * half + 512],
                    pwbufs[p][half][:],
                    AF.Tanh,
                    scale=inv_scale,
                    **kw,
                )

            def emit_out(p):
                # transposed: out rows = hidden sub-dim (M=128), col pair =
                # the two steps; 32 pairs accumulate in one PSUM bank
                wa = wabufs.pop(p)
                base = 16 * (p % 32)
                for i in range(8):
                    nc.tensor.matmul(
                        poT[:, base + 2 * i : base + 2 * i + 2],
                        wa[:, 128 * i : 128 * i + 128],
                        xTp_sb[:, 2 * p : 2 * p + 2],
                        start=True,
                        stop=True,
                    )
                if p % 32 == 31:
                    so = stg.tile([128, 512], F32, tag="so")
                    pending_so.append((p // 32, so, 0))
                    pending_so.append((p // 32, so, 1))

            emit_wx(0, closed=True)
            for t in range(T):
                j = t % 2  # position within the step pair
                p = t // 2
                if j == 0:
                    h16bufs[p] = hsp.tile(
                        [POS_EM_DIM, 2 * SH], F16, name=f"h16p{p}", tag="h16p"
                    )
                # ---- PE: the four Whh chain matmuls, closing each
                # bank's prefilled accumulation group ----
                if t > 0:
                    emit_whh(t, p, j)
                # ---- attention tail: kT -> kcopy -> w for pair p-1;
                # kcopy lands in the DVE-idle Whh+sigma window, the w
                # matmuls run once the copy's sem arrives ----
                if j == 0 and p >= 1:
                    emit_kT(p - 1)
                    emit_kcopy(p - 1)
                    emit_w(p - 1)
                # ---- Act: one sigmoid for all four gates ----
                sig = CB[:, 0:256]
                sig_out = sig.rearrange("p (b x) -> p b x", b=4)
                nc.scalar.activation(sig_out, sig_in_view, AF.Sigmoid)
                # ---- PE: prefill Wx for t+1; the WAR on sigma(t) holds
                # it until the gates are read, then it runs inside this
                # step's chain window ----
                if t + 1 < T:
                    emit_wx(t + 1)
                # ---- DVE: c update  (tanh(g) = 2*sigmoid(2g) - 1) ----
                # one paged mul: page0 t1 = sig_i*sig_g2, page1 cmid = sig_f*c
                t1cm = work.tile([128, 128], F32, tag="t1cm")
                in0 = CB[:, 0:128].rearrange("p (b x) -> p b x", b=2)
                in1 = CB[:, 128:320].rearrange("p (b x) -> p b x", b=3)[:, 0::2, :]
                nc.vector.tensor_mul(
                    t1cm[:].rearrange("p (b x) -> p b x", b=2), in0, in1
                )
                u = work.tile([128, SH], F32, tag="u")
                nc.vector.scalar_tensor_tensor(
                    u[:], t1cm[:, 0:64], 2.0, sig[:, 0:64], ALU.mult, ALU.subtract
                )
                nc.vector.tensor_add(cT, t1cm[:, 64:128], u[:])
                # filler copy reading through cT: ready exactly at cadd,
                # sized to end just past tanh(c) so the h16 multiply
                # dispatches from a busy engine (no idle-wakeup)
                fil = aux.tile([128, 480], F32, tag="fil")
                nc.vector.tensor_copy(fil[:], CB[:, 100:580])
                # ---- Act: lagged wa chunk fills the sigma->tanh gap ----
                if j == 0 and p >= 2:
                    emit_wa_chunk(p - 2, 1, gate=sig[:, 0:1])
                elif j == 1 and p >= 1:
                    emit_wa_chunk(p - 1, 0, gate=sig[:, 0:1])
                # ---- Act: tanh(c);  DVE: h16 = sig_o * tanh(c) ----
                tct = work.tile([128, SH], F32, tag="tct")
                nc.scalar.activation(tct[:], cT, AF.Tanh)
                nc.vector.tensor_mul(
                    h16bufs[p][:, SH * j : SH * j + SH], sig[:, 192:256], tct[:]
                )
                # last step: pull pair-126's second chunk out of the
                # epilogue into the (now tail-less) post-tanh window
                if t == T - 1:
                    emit_wa_chunk(p - 1, 1, gate=cT[:, 0:1])
                # ---- tails: out staging halves from earlier boundaries
                # drain first (so a fresh boundary's halves pop one step
                # later, when their poT deps have long arrived) ----
                if pending_so:
                    emit_so_half(fil[:, 0:256])
                if j == 0:
                    if p >= 2:
                        emit_out(p - 2)
                    h16bufs.pop(p - 2, None)
                # ---- PE pacer: dummy matmuls (into unused pk-bank cols)
                # keep PE busy until h16's sem has landed, so Whh(t+1)
                # dispatches without the blocked-redispatch penalty.
                # Gates ride the moving operand: after a j=0 step PE is
                # busy long past cadd, so gate on sigma only (CB[:,0:256])
                # and run long; after a j=1 step PE idles early, so gate
                # on cT (arrives while PE still busy) and run short.
                if t + 1 < T:
                    if j == 1:
                        nc.tensor.matmul(
                            pk_t[0:1, 128:296], dum_st[:], CB[:, 152:320],
                            start=True, stop=True,
                        )
                    else:
                        # PE goes idle right after the out matmuls here; a
                        # tiny pacer bridges to h16's sem arrival
                        nc.tensor.matmul(
                            pk_t[0:1, 128:168], dum_st[:], CB[:, 280:320],
                            start=True, stop=True,
                        )
            # ---- epilogue: drain the attention pipeline ----
            while pending_so:
                emit_so_half()
            last = T // 2 - 1  # 127
            emit_kT(last)
            emit_kcopy(last)
            emit_w(last)
            emit_out(last - 1)
            emit_wa_chunk(last, 0)
            emit_wa_chunk(last, 1)
            emit_out(last)
            while pending_so:
                emit_so_half()
    if split:
        nc.m = _split_multi_waits(nc.m)
    _NC_CACHE[split] = nc
    return nc


def kernel(
    obs,
    prev_act,
    in_shift,
    in_scale,
    pos_embedding,
    W_ih,
    b_ih,
    W_hh,
    b_hh,
    Wq,
    bq,
    Wk,
    bk,
):
    obs = np.asarray(obs, np.float32)
    prev_act = np.asarray(prev_act, np.float32)
    in_shift = np.asarray(in_shift, np.float32)
    in_scale = np.asarray(in_scale, np.float32)
    pos_embedding = np.asarray(pos_embedding, np.float32)
    W_ih = np.asarray(W_ih, np.float32)
    b_ih = np.asarray(b_ih, np.float32)
    W_hh = np.asarray(W_hh, np.float32)
    b_hh = np.asarray(b_hh, np.float32)
    Wq = np.asarray(Wq, np.float32)
    bq = np.asarray(bq, np.float32)
    Wk = np.asarray(Wk, np.float32)
    bk = np.asarray(bk, np.float32)

    x = (obs - in_shift) / (in_scale + 1e-8)  # (T, 512)
    q = pos_embedding @ Wq.T + bq  # (1024, 128)
    qT = np.ascontiguousarray(q.T)  # (128, 1024)

    def blocks(mat_rows):  # reorder gate blocks to [i, f, g, o]
        return np.concatenate([mat_rows[128 * p : 128 * p + 128] for p in _PERM], 0)

    W_ih_r = blocks(W_ih)  # (512, 33)
    W_hh_r = blocks(W_hh)  # (512, 128)
    b_r = blocks((b_ih + b_hh)[:, None])[:, 0]  # (512,)
    # g block (rows 256:384 after reorder) doubled: tanh(g) = 2*sig(2g)-1
    gs = np.ones((512, 1), np.float32)
    gs[256:384] = 2.0
    W_ih_r = W_ih_r * gs
    W_hh_r = W_hh_r * gs
    b_r = b_r * gs[:, 0]

    whhT = np.ascontiguousarray(W_hh_r.T)  # (128, 512)
    wih1T = np.concatenate(
        [W_ih_r[:, 0:1].T, np.ascontiguousarray(W_ih_r[:, 1:33].T), b_r[None, :]], 0
    )  # (34, 512)
    # fp16 hi/lo folded into K: [Whi; Whi; Wlo] x [xhi; xlo; xhi] gives
    # Whi*xhi + Whi*xlo + Wlo*xhi (residual ~2^-22) in one K=102 fp16 MM
    whi = wih1T.astype(np.float16)
    wlo = (wih1T - whi.astype(np.float32)).astype(np.float16)
    wih3T = np.concatenate([whi, whi, wlo], 0)  # (102, 512) fp16
    wkT = np.ascontiguousarray(Wk.T)  # (128, 128)
    bkr = bk[None, :]  # (1, 128)

    nc = _build_nc()
    shared = {
        "whhT": whhT.astype(np.float16),
        "wih3T": np.ascontiguousarray(wih3T),
        "wkT": wkT.astype(np.float16),
        "bkr": np.ascontiguousarray(bkr).astype(np.float16),
        "qT": qT.astype(np.float16),
    }
    in_maps = []
    for c in range(NCORES):
        xs = x[:, c * SH : (c + 1) * SH]  # (T, 64)
        xa1 = np.empty((34, T * SH), np.float32)
        xa1[0] = xs.reshape(-1)
        xa1[1:33] = np.repeat(prev_act.T, SH, axis=1).reshape(32, T * SH)
        xa1[33] = 1.0
        xahi = xa1.astype(np.float16)
        xalo = (xa1 - xahi.astype(np.float32)).astype(np.float16)
        xa3 = np.concatenate([xahi, xalo, xahi], 0)  # (102, T*SH) fp16
        # block-diagonal paired s columns: col t has s_t in rows [64j, 64j+64)
        # for j = t%2, zeros elsewhere
        xTp = np.zeros((2 * SH, T), np.float16)
        xTp[0:SH, 0::2] = xs.T[:, 0::2]
        xTp[SH : 2 * SH, 1::2] = xs.T[:, 1::2]
        in_maps.append({**shared, "xa3": xa3, "xTp": xTp})

    res = run_bass_kernel_spmd(nc, in_maps, list(range(NCORES)), trace=TRACE[0])
    LAST_RESULTS[0] = res
    total = np.zeros((T, HIDDEN_DIM), np.float32)
    for c in range(NCORES):
        raw = res.results[c]["outs"]  # (T//64, 128, 512)
        # col = (pair%32)*16 + hidden_block*2 + step_in_pair
        total += np.transpose(
            raw.reshape(T // 64, 128, 32, 8, 2), (0, 2, 4, 3, 1)
        ).reshape(T, HIDDEN_DIM)
    return np.tanh(total).astype(np.float32)
